# revision 17
# baseline (speedup 1.0000x reference)
"""2-layer GAT (graph attention) Bass/Tile kernel for Trainium2, 8-core SPMD.

Sharding: nodes partitioned contiguously across cores; edges assigned to the
core owning their dst, sorted by dst, grouped into 128-dst blocks and 128-edge
chunks (uniformly padded so all cores share one SPMD module).

Per core: build the full node feature table (replicated matmul from a
host-transposed xT), rows [feat bf16 | el bf16 | pad] in HBM (256B-multiple
rows for dma_gather).  Edge phase per block: dma_gather of src rows (split
into lo/hi index halves since gather indices are int16), a dense per-block er
load broadcast to edges via the PE-transposed one-hot, then per chunk one PE
matmul accumulating the weighted message sum and the exp-sum into PSUM
(softmax max-subtraction is skipped; |e| is O(1)).  Per-dst normalization
happens after the reduction.  Hidden states are transposed and AllGather'ed
between layers.
"""

import math

import numpy as np

import concourse.bacc as bacc
import concourse.bass as bass
import concourse.mybir as mybir
import concourse.tile as tile
from concourse.masks import make_identity

F32 = mybir.dt.float32
BF16 = mybir.dt.bfloat16
I32 = mybir.dt.int32
I16 = mybir.dt.int16
AF = mybir.ActivationFunctionType
OP = mybir.AluOpType

P = 128
HALF = 32768  # int16 gather index limit


class GATCfg:
    def __init__(self, N=50000, C=8, IN=128, HID=32, HEADS=8, OUT=16, NEG=0.2):
        self.N, self.C, self.IN = N, C, IN
        self.HID, self.HEADS, self.OUT, self.NEG = HID, HEADS, OUT, NEG
        self.HALF = HALF   # int16 gather index split (overridable in tests)
        self.SLO = 0   # lo-half slots per block (set by prep)
        self.SHI = 0   # hi-half slots per block
        self.F1 = HEADS * HID
        self.F2 = HEADS * OUT
        # table rows padded to a 256B multiple (bf16 elements)
        self.ROW1 = 384 if self.F1 + 8 > 256 else 256      # 768B
        self.ROW2 = 256 if self.F2 + 8 > 128 else 128      # 512B
        self.Nloc = (N + C - 1) // C
        self.NB = (self.Nloc + P - 1) // P
        self.Nlp = self.NB * P
        self.NP1 = ((N + 511) // 512) * 512
        self.NP2 = C * self.Nlp

    @property
    def NCt(self):
        return (self.SLO + self.SHI) // P


def _wrap16(vals_slots, NB, S):
    """[NB, S] slot-ordered ints -> [128, NB*S//16] 16-wrapped, replicated 8x."""
    a = vals_slots.reshape(NB, S // 16, 16)
    out = np.zeros((128, NB * (S // 16)), np.int16)
    for b in range(NB):
        blkcols = a[b].T.astype(np.int16)          # [16, S//16]
        for r in range(8):
            out[r * 16:(r + 1) * 16,
                b * (S // 16):(b + 1) * (S // 16)] = blkcols
    return out


def prep_indices(src, dst, cfg):
    """Host index-only preprocessing."""
    C, Nloc, NB = cfg.C, cfg.Nloc, cfg.NB
    src = np.asarray(src).astype(np.int64)
    dst = np.asarray(dst).astype(np.int64)
    core = dst // Nloc
    dloc = dst - core * Nloc
    blk = dloc // P
    dblk = dloc - blk * P
    key = core * NB + blk
    order = np.argsort(key, kind="stable")
    counts = np.bincount(key, minlength=C * NB)
    starts = np.zeros(C * NB + 1, np.int64)
    np.cumsum(counts, out=starts[1:])

    r_of = src // Nloc
    srcp = r_of * cfg.Nlp + (src - r_of * Nloc)   # layer-2 rank-major id

    # per-layer lo/hi counts to size SLO/SHI uniformly
    HALFc = cfg.HALF

    def lohi_max(ids):
        lo_max = hi_max = 0
        for k in range(C * NB):
            e = order[starts[k]:starts[k + 1]]
            n_lo = int((ids[e] < HALFc).sum())
            lo_max = max(lo_max, n_lo)
            hi_max = max(hi_max, e.size - n_lo)
        return lo_max, hi_max

    lo1, hi1 = lohi_max(src)
    lo2, hi2 = lohi_max(srcp)
    SLO = ((max(lo1, lo2, 1) + P - 1) // P) * P
    SHI = ((max(hi1, hi2) + P - 1) // P) * P
    cfg.SLO, cfg.SHI = SLO, SHI
    S = SLO + SHI
    NCt = S // P

    out = {}
    for layer, ids in ((1, src), (2, srcp)):
        idxlo = np.zeros((C, NB, SLO), np.int64)
        idxhi = np.zeros((C, NB, max(SHI, 16)), np.int64)
        dstb = np.full((C, P, NB * NCt), -1.0, np.float32)
        for c in range(C):
            for b in range(NB):
                k = c * NB + b
                e = order[starts[k]:starts[k + 1]]
                v = ids[e]
                m = v < HALFc
                elo, ehi = e[m], e[~m]
                idxlo[c, b, :elo.size] = v[m]
                if ehi.size:
                    idxhi[c, b, :ehi.size] = v[~m] - HALFc
                # dst-in-block values at slots [0,nlo) and [SLO, SLO+nhi)
                sl = np.concatenate([np.arange(elo.size),
                                     SLO + np.arange(ehi.size)])
                ee = np.concatenate([elo, ehi])
                dstb[c, sl % P, b * NCt + sl // P] = dblk[ee]
        wlo = np.stack([_wrap16(idxlo[c], NB, SLO) for c in range(C)])
        whi = np.stack([_wrap16(idxhi[c], NB, max(SHI, 16)) for c in range(C)])
        out[f"idxlo{layer}"] = wlo
        out[f"idxhi{layer}"] = whi
        out[f"dstb{layer}"] = dstb
    return out


def _alr_mat(al, ar, F, H, D):
    m = np.zeros((F, 16), np.float32)
    for h in range(H):
        m[h * D:(h + 1) * D, h] = al[h]
        m[h * D:(h + 1) * D, 8 + h] = ar[h]
    return m


def host_inputs(inputs, cfg, idx):
    x = np.asarray(inputs["x"], np.float32)
    xT = np.zeros((cfg.IN, cfg.NP1), np.float32)
    xT[:, :cfg.N] = np.ascontiguousarray(x.T)
    W1 = np.asarray(inputs["W1"], np.float32)
    W2 = np.asarray(inputs["W2"], np.float32)
    alr1 = _alr_mat(np.asarray(inputs["al1"], np.float32),
                    np.asarray(inputs["ar1"], np.float32),
                    cfg.F1, cfg.HEADS, cfg.HID)
    alr2 = _alr_mat(np.asarray(inputs["al2"], np.float32),
                    np.asarray(inputs["ar2"], np.float32),
                    cfg.F2, cfg.HEADS, cfg.OUT)
    b1 = np.asarray(inputs["b1"], np.float32).reshape(1, cfg.F1)
    b2 = np.asarray(inputs["b2"], np.float32).reshape(1, cfg.F2)

    in_maps = []
    for c in range(cfg.C):
        xTl = np.zeros((cfg.IN, cfg.Nlp), np.float32)
        lo = c * cfg.Nloc
        hi = min(cfg.N, lo + cfg.Nloc)
        xTl[:, :hi - lo] = xT[:, lo:hi]
        in_maps.append({
            "xT": xT, "xTl": xTl,
            "W1": W1, "W1T": np.ascontiguousarray(W1.T),
            "W2": W2, "W2T": np.ascontiguousarray(W2.T),
            "ALR1": alr1, "ALR2": alr2,
            "B1": b1, "B2": b2,
            "idxlo1": idx["idxlo1"][c], "idxhi1": idx["idxhi1"][c],
            "idxlo2": idx["idxlo2"][c], "idxhi2": idx["idxhi2"][c],
            "dstb1": idx["dstb1"][c], "dstb2": idx["dstb2"][c],
        })
    return in_maps


def build_module(cfg, dbg=False, skip_cc=False):
    nc = bacc.Bacc("TRN2", target_bir_lowering=False, debug=False,
                   num_devices=cfg.C)
    C, NB, Nlp = cfg.C, cfg.NB, cfg.Nlp
    F1, F2, ROW1, ROW2 = cfg.F1, cfg.F2, cfg.ROW1, cfg.ROW2
    SLO, SHI, NCt = cfg.SLO, cfg.SHI, cfg.NCt
    NLOC, NHIC = SLO // P, SHI // P

    d_xT = nc.dram_tensor("xT", [cfg.IN, cfg.NP1], F32, kind="ExternalInput")
    d_xTl = nc.dram_tensor("xTl", [cfg.IN, Nlp], F32, kind="ExternalInput")
    d_W1 = nc.dram_tensor("W1", [cfg.IN, F1], F32, kind="ExternalInput")
    d_W1T = nc.dram_tensor("W1T", [F1, cfg.IN], F32, kind="ExternalInput")
    d_W2 = nc.dram_tensor("W2", [F1, F2], F32, kind="ExternalInput")
    d_W2T = nc.dram_tensor("W2T", [F2, F1], F32, kind="ExternalInput")
    d_ALR1 = nc.dram_tensor("ALR1", [F1, 16], F32, kind="ExternalInput")
    d_ALR2 = nc.dram_tensor("ALR2", [F2, 16], F32, kind="ExternalInput")
    d_B1 = nc.dram_tensor("B1", [1, F1], F32, kind="ExternalInput")
    d_B2 = nc.dram_tensor("B2", [1, F2], F32, kind="ExternalInput")
    d_il1 = nc.dram_tensor("idxlo1", [P, NB * (SLO // 16)], I16,
                           kind="ExternalInput")
    d_ih1 = nc.dram_tensor("idxhi1", [P, NB * (max(SHI, 16) // 16)], I16,
                           kind="ExternalInput")
    d_il2 = nc.dram_tensor("idxlo2", [P, NB * (SLO // 16)], I16,
                           kind="ExternalInput")
    d_ih2 = nc.dram_tensor("idxhi2", [P, NB * (max(SHI, 16) // 16)], I16,
                           kind="ExternalInput")
    d_db1 = nc.dram_tensor("dstb1", [P, NB * NCt], F32, kind="ExternalInput")
    d_db2 = nc.dram_tensor("dstb2", [P, NB * NCt], F32, kind="ExternalInput")
    d_out = nc.dram_tensor("out", [cfg.Nloc, cfg.OUT], F32,
                           kind="ExternalOutput")

    d_tab1 = nc.dram_tensor("tab1", [cfg.NP1, ROW1], BF16, kind="Internal")
    d_tab2 = nc.dram_tensor("tab2", [cfg.NP2, ROW2], BF16, kind="Internal")
    d_er1 = nc.dram_tensor("er1", [Nlp, 8], BF16, kind="Internal")
    d_er2 = nc.dram_tensor("er2", [Nlp, 8], BF16, kind="Internal")
    d_hT = nc.dram_tensor("hT", [2, P, Nlp], BF16, kind="Internal")
    d_hTg = None
    if C > 1:
        d_hTg = nc.dram_tensor("hTg", [C, 2, P, Nlp], BF16, kind="Internal",
                               addr_space="Shared" if C > 4 else "Local")

    with tile.TileContext(nc) as tc:
        with (
            tc.tile_pool(name="const", bufs=1) as cpool,
            tc.tile_pool(name="work", bufs=3) as wpool,
            tc.tile_pool(name="gath", bufs=2) as gpool,
        ):
            # ---------------- constants ----------------
            iota_i = cpool.tile([P, P], I32)
            nc.gpsimd.iota(iota_i[:], pattern=[[1, P]], base=0,
                           channel_multiplier=0)
            iota_bf = cpool.tile([P, P], BF16)
            nc.vector.tensor_copy(iota_bf[:], iota_i[:])
            ident = cpool.tile([P, P], BF16)
            make_identity(nc, ident[:])
            ones1 = cpool.tile([1, P], BF16)
            nc.vector.memset(ones1[:], 1.0)

            rhs1 = cpool.tile([P, F1 + 16], BF16)
            rhs2 = cpool.tile([P, 2, F2 + 16], BF16)
            b1row = cpool.tile([P, F1], F32)
            b2mean = cpool.tile([P, cfg.OUT], F32)

            with tc.tile_pool(name="cps", bufs=2, space="PSUM") as cps:
                w1t_a = cpool.tile([P, cfg.IN], F32, name="w1t_a")
                w1t_b = cpool.tile([P, cfg.IN], F32, name="w1t_b")
                nc.sync.dma_start(w1t_a[:], d_W1T[0:P, :])
                nc.sync.dma_start(w1t_b[:], d_W1T[P:2 * P, :])
                alr1_s = cpool.tile([P, 2, 16], F32)
                nc.sync.dma_start(alr1_s[:, 0, :], d_ALR1[0:P, :])
                nc.sync.dma_start(alr1_s[:, 1, :], d_ALR1[P:F1, :])
                ps_wel = cps.tile([P, 16], F32, tag="cwel")
                nc.tensor.matmul(ps_wel[:], lhsT=w1t_a[:], rhs=alr1_s[:, 0, :],
                                 start=True, stop=False)
                nc.tensor.matmul(ps_wel[:], lhsT=w1t_b[:], rhs=alr1_s[:, 1, :],
                                 start=False, stop=True)
                w1_f = cpool.tile([P, F1], F32, name="w1_f")
                nc.sync.dma_start(w1_f[:], d_W1[:, :])
                nc.vector.tensor_copy(rhs1[:, 0:F1], w1_f[:])
                nc.vector.tensor_copy(rhs1[:, F1:F1 + 16], ps_wel[:])

                w2t_s = cpool.tile([F2, F1], F32)
                nc.sync.dma_start(w2t_s[:], d_W2T[:, :])
                alr2_s = cpool.tile([F2, 16], F32)
                nc.sync.dma_start(alr2_s[:], d_ALR2[:, :])
                ps_wel2 = cps.tile([P, 2, 16], F32, tag="cwel2")
                for q in range(2):
                    nc.tensor.matmul(ps_wel2[:, q, :],
                                     lhsT=w2t_s[:, q * P:(q + 1) * P],
                                     rhs=alr2_s[:], start=True, stop=True)
                w2_f = cpool.tile([P, 2, F2], F32)
                nc.sync.dma_start(w2_f[:, 0, :], d_W2[0:P, :])
                nc.sync.dma_start(w2_f[:, 1, :], d_W2[P:F1, :])
                for q in range(2):
                    nc.vector.tensor_copy(rhs2[:, q, 0:F2], w2_f[:, q, :])
                    nc.vector.tensor_copy(rhs2[:, q, F2:F2 + 16],
                                          ps_wel2[:, q, :])

                b1_r = cpool.tile([1, F1], BF16, name="b1_r")
                b1_f = cpool.tile([1, F1], F32, name="b1_f")
                nc.sync.dma_start(b1_f[:], d_B1[:, :])
                nc.vector.tensor_copy(b1_r[:], b1_f[:])
                ps_b1 = cps.tile([P, F1], F32, tag="cb1")
                nc.tensor.matmul(ps_b1[:], lhsT=ones1[:], rhs=b1_r[:],
                                 start=True, stop=True)
                nc.vector.tensor_copy(b1row[:], ps_b1[:])

                b2_r = cpool.tile([1, F2], BF16, name="b2_r")
                b2_f = cpool.tile([1, F2], F32, name="b2_f")
                nc.sync.dma_start(b2_f[:], d_B2[:, :])
                nc.vector.tensor_copy(b2_r[:], b2_f[:])
                ps_b2 = cps.tile([P, F2], F32, tag="cb2")
                nc.tensor.matmul(ps_b2[:], lhsT=ones1[:], rhs=b2_r[:],
                                 start=True, stop=True)
                b2full = cpool.tile([P, F2], F32)
                nc.vector.tensor_copy(b2full[:], ps_b2[:])
                b2h = cpool.tile([P, F2 // 2], F32)
                nc.vector.tensor_add(b2h[:], b2full[:, 0:F2 // 2],
                                     b2full[:, F2 // 2:F2])
                b2q = cpool.tile([P, F2 // 4], F32)
                nc.vector.tensor_add(b2q[:], b2h[:, 0:F2 // 4],
                                     b2h[:, F2 // 4:F2 // 2])
                b2s = cpool.tile([P, cfg.OUT], F32)
                nc.vector.tensor_add(b2s[:], b2q[:, 0:cfg.OUT],
                                     b2q[:, cfg.OUT:2 * cfg.OUT])
                nc.scalar.mul(b2mean[:], b2s[:], 0.125)

            # ---------------- layer-1 table ----------------
            with tc.tile_pool(name="t1ps", bufs=3, space="PSUM") as t1ps:
                nt1 = cfg.NP1 // P
                for t in range(nt1):
                    xt = wpool.tile([P, P], F32, tag="xt")
                    nc.sync.dma_start(xt[:], d_xT[:, t * P:(t + 1) * P])
                    xb = wpool.tile([P, P], BF16, tag="xb")
                    (nc.vector.tensor_copy if t % 2 == 0
                     else nc.scalar.copy)(xb[:], xt[:])
                    ps = t1ps.tile([P, F1 + 16], F32, tag="tbl")
                    nc.tensor.matmul(ps[:], lhsT=xb[:], rhs=rhs1[:],
                                     start=True, stop=True)
                    fe = wpool.tile([P, F1 + 8], BF16, tag="fe")
                    (nc.scalar.copy if t % 2 == 0
                     else nc.vector.tensor_copy)(fe[:], ps[:, 0:F1 + 8])
                    nc.sync.dma_start(d_tab1[t * P:(t + 1) * P, 0:F1 + 8],
                                      fe[:])

                for t in range(NB):
                    xt = wpool.tile([P, P], F32, tag="xt")
                    nc.sync.dma_start(xt[:], d_xTl[:, t * P:(t + 1) * P])
                    xb = wpool.tile([P, P], BF16, tag="xb")
                    nc.vector.tensor_copy(xb[:], xt[:])
                    ps = t1ps.tile([P, 16], F32, tag="er")
                    nc.tensor.matmul(ps[:], lhsT=xb[:],
                                     rhs=rhs1[:, F1:F1 + 16],
                                     start=True, stop=True)
                    erb = wpool.tile([P, 8], BF16, tag="erb")
                    nc.vector.tensor_copy(erb[:], ps[:, 8:16])
                    nc.sync.dma_start(d_er1[t * P:(t + 1) * P, :], erb[:])

            # ---------------- edge phase (shared) ----------------
            def edge_phase(layer, pspool, tps):
                F = F1 if layer == 1 else F2
                ROW = ROW1 if layer == 1 else ROW2
                tab = d_tab1 if layer == 1 else d_tab2
                ertab = d_er1 if layer == 1 else d_er2
                d_il = d_il1 if layer == 1 else d_il2
                d_ih = d_ih1 if layer == 1 else d_ih2
                d_db = d_db1 if layer == 1 else d_db2
                CL = SLO // 16
                CH = max(SHI, 16) // 16
                PIECE = 768  # max slots per dma_gather (64-desc/lane packet cap)
                for b in range(NB):
                    il = wpool.tile([P, CL], I16, tag="il")
                    nc.sync.dma_start(il[:], d_il[:, b * CL:(b + 1) * CL])
                    Glo = gpool.tile([P, NLOC, ROW], BF16, tag=f"Glo{layer}")
                    for s0 in range(0, SLO, PIECE):
                        n = min(PIECE, SLO - s0)
                        nc.gpsimd.dma_gather(
                            out_ap=Glo[:, s0 // P:(s0 + n) // P, :],
                            in_ap=tab[:, :],
                            idxs_ap=il[:, s0 // 16:(s0 + n) // 16],
                            num_idxs=n, num_idxs_reg=n, elem_size=ROW)
                    if SHI > 0:
                        ih = wpool.tile([P, CH], I16, tag="ih")
                        nc.sync.dma_start(ih[:], d_ih[:, b * CH:(b + 1) * CH])
                        Ghi = gpool.tile([P, NHIC, ROW], BF16,
                                         tag=f"Ghi{layer}")
                        for s0 in range(0, SHI, PIECE):
                            n = min(PIECE, SHI - s0)
                            nc.gpsimd.dma_gather(
                                out_ap=Ghi[:, s0 // P:(s0 + n) // P, :],
                                in_ap=tab[cfg.HALF:, :],
                                idxs_ap=ih[:, s0 // 16:(s0 + n) // 16],
                                num_idxs=n, num_idxs_reg=n, elem_size=ROW)
                    db = wpool.tile([P, NCt], F32, tag="db")
                    nc.sync.dma_start(db[:], d_db[:, b * NCt:(b + 1) * NCt])
                    erblk = wpool.tile([P, 8], BF16, tag="erblk")
                    nc.sync.dma_start(erblk[:], ertab[b * P:(b + 1) * P, :])

                    ps = pspool.tile([P, F + 8], F32, tag="eps")
                    for c in range(NCt):
                        Gc = (Glo[:, c, :] if c < NLOC
                              else Ghi[:, c - NLOC, :])
                        oh = wpool.tile([P, P], BF16, tag="oh")
                        nc.vector.tensor_scalar(oh[:], iota_bf[:],
                                                db[:, c:c + 1], None,
                                                op0=OP.is_equal)
                        pst = tps.tile([P, P], BF16, tag="psoh")
                        nc.tensor.transpose(pst[:], oh[:], ident[:])
                        ohT = wpool.tile([P, P], BF16, tag="ohT")
                        nc.scalar.copy(ohT[:], pst[:])
                        erps = tps.tile([P, 8], F32, tag="erps")
                        nc.tensor.matmul(erps[:], lhsT=ohT[:], rhs=erblk[:],
                                         start=True, stop=True)
                        e = wpool.tile([P, 8], F32, tag="e")
                        nc.vector.tensor_tensor(out=e[:], in0=Gc[:, F:F + 8],
                                                in1=erps[:], op=OP.add)
                        es = wpool.tile([P, 8], F32, tag="es")
                        nc.vector.tensor_scalar(es[:], e[:], cfg.NEG, None,
                                                op0=OP.mult)
                        e2 = wpool.tile([P, 8], F32, tag="e2")
                        nc.vector.tensor_tensor(out=e2[:], in0=e[:],
                                                in1=es[:], op=OP.max)
                        msg = wpool.tile([P, F + 8], BF16, tag="msg")
                        nc.scalar.activation(msg[:, F:F + 8], e2[:], AF.Exp)
                        ex_b = msg[:, F:F + 8].rearrange(
                            "p (h one) -> p h one", one=1)
                        nc.vector.tensor_tensor(
                            out=msg[:, 0:F].rearrange("p (h d) -> p h d", h=8),
                            in0=Gc[:, 0:F].rearrange("p (h d) -> p h d", h=8),
                            in1=ex_b.to_broadcast([P, 8, F // 8]),
                            op=OP.mult)
                        nc.tensor.matmul(ps[:], lhsT=oh[:], rhs=msg[:],
                                         start=(c == 0), stop=(c == NCt - 1))

                    esum = wpool.tile([P, 8], F32, tag="esum")
                    nc.vector.tensor_scalar(esum[:], ps[:, F:F + 8], 1e-30,
                                            None, op0=OP.max)
                    inv = wpool.tile([P, 8], F32, tag="inv")
                    nc.vector.reciprocal(inv[:], esum[:])
                    yield b, ps, inv

            # ---------------- layer-1 edges + hT ----------------
            with tc.tile_pool(name="e1ps", bufs=2, space="PSUM") as e1ps, \
                 tc.tile_pool(name="tps", bufs=2, space="PSUM") as tps:
                for b, ps, inv in edge_phase(1, e1ps, tps):
                    z = wpool.tile([P, F1], F32, tag="z")
                    nc.vector.tensor_tensor(
                        out=z[:].rearrange("p (h d) -> p h d", h=8),
                        in0=ps[:, 0:F1].rearrange("p (h d) -> p h d", h=8),
                        in1=inv[:].rearrange("p (h one) -> p h one", one=1)
                            .to_broadcast([P, 8, cfg.HID]),
                        op=OP.mult)
                    nc.vector.tensor_add(z[:], z[:], b1row[:])
                    zm = wpool.tile([P, F1], F32, tag="zm")
                    nc.vector.tensor_scalar(zm[:], z[:], 0.0, None, op0=OP.min)
                    zp = wpool.tile([P, F1], F32, tag="zp")
                    nc.vector.tensor_scalar(zp[:], z[:], 0.0, None, op0=OP.max)
                    q_ = wpool.tile([P, F1], F32, tag="q_")
                    nc.scalar.activation(q_[:], zm[:], AF.Exp)
                    s_ = wpool.tile([P, F1], F32, tag="s_")
                    nc.vector.tensor_add(s_[:], zp[:], q_[:])
                    hb = wpool.tile([P, F1], BF16, tag="hb")
                    nc.vector.tensor_scalar(hb[:], s_[:], -1.0, None,
                                            op0=OP.add)
                    for q in range(2):
                        pst = tps.tile([P, P], BF16, tag="pst")
                        nc.tensor.transpose(pst[:], hb[:, q * P:(q + 1) * P],
                                            ident[:])
                        htp = wpool.tile([P, P], BF16, tag="htp")
                        (nc.vector.tensor_copy if q == 0
                         else nc.scalar.copy)(htp[:], pst[:])
                        nc.sync.dma_start(d_hT[q, :, b * P:(b + 1) * P],
                                          htp[:])

            # ---------------- allgather ----------------
            if C > 1 and not skip_cc:
                nc.gpsimd.collective_compute(
                    "AllGather", OP.bypass,
                    replica_groups=[list(range(C))],
                    ins=[d_hT[:, :, :]],
                    outs=[d_hTg[:, :, :, :]],
                )

            # ---------------- layer-2 table ----------------
            with tc.tile_pool(name="t2ps", bufs=3, space="PSUM") as t2ps:
                for r in range(C):
                    for t in range(NB):
                        ht_a = wpool.tile([P, P], BF16, tag="ht_a")
                        ht_b = wpool.tile([P, P], BF16, tag="ht_b")
                        if C > 1:
                            nc.sync.dma_start(
                                ht_a[:], d_hTg[r, 0, :, t * P:(t + 1) * P])
                            nc.sync.dma_start(
                                ht_b[:], d_hTg[r, 1, :, t * P:(t + 1) * P])
                        else:
                            nc.sync.dma_start(ht_a[:],
                                              d_hT[0, :, t * P:(t + 1) * P])
                            nc.sync.dma_start(ht_b[:],
                                              d_hT[1, :, t * P:(t + 1) * P])
                        ps = t2ps.tile([P, F2 + 16], F32, tag="tbl2")
                        nc.tensor.matmul(ps[:], lhsT=ht_a[:], rhs=rhs2[:, 0, :],
                                         start=True, stop=False)
                        nc.tensor.matmul(ps[:], lhsT=ht_b[:], rhs=rhs2[:, 1, :],
                                         start=False, stop=True)
                        fe = wpool.tile([P, F2 + 8], BF16, tag="fe2")
                        (nc.scalar.copy if t % 2 == 0
                         else nc.vector.tensor_copy)(fe[:], ps[:, 0:F2 + 8])
                        row0 = (r * NB + t) * P
                        nc.sync.dma_start(d_tab2[row0:row0 + P, 0:F2 + 8],
                                          fe[:])

                for t in range(NB):
                    ht_a = wpool.tile([P, P], BF16, tag="ht_a")
                    nc.sync.dma_start(ht_a[:], d_hT[0, :, t * P:(t + 1) * P])
                    ht_b = wpool.tile([P, P], BF16, tag="ht_b")
                    nc.sync.dma_start(ht_b[:], d_hT[1, :, t * P:(t + 1) * P])
                    ps = t2ps.tile([P, 16], F32, tag="er")
                    nc.tensor.matmul(ps[:], lhsT=ht_a[:],
                                     rhs=rhs2[:, 0, F2:F2 + 16],
                                     start=True, stop=False)
                    nc.tensor.matmul(ps[:], lhsT=ht_b[:],
                                     rhs=rhs2[:, 1, F2:F2 + 16],
                                     start=False, stop=True)
                    erb = wpool.tile([P, 8], BF16, tag="erb")
                    nc.vector.tensor_copy(erb[:], ps[:, 8:16])
                    nc.sync.dma_start(d_er2[t * P:(t + 1) * P, :], erb[:])

            # ---------------- layer-2 edges + output ----------------
            OUTW = cfg.OUT
            with tc.tile_pool(name="e2ps", bufs=2, space="PSUM") as e2ps, \
                 tc.tile_pool(name="tps2", bufs=2, space="PSUM") as tps2:
                for b, ps, inv in edge_phase(2, e2ps, tps2):
                    inv8 = wpool.tile([P, 8], F32, tag="inv8")
                    nc.scalar.mul(inv8[:], inv[:], 0.125)
                    w_ = wpool.tile([P, F2], F32, tag="w_")
                    nc.vector.tensor_tensor(
                        out=w_[:].rearrange("p (h d) -> p h d", h=8),
                        in0=ps[:, 0:F2].rearrange("p (h d) -> p h d", h=8),
                        in1=inv8[:].rearrange("p (h one) -> p h one", one=1)
                            .to_broadcast([P, 8, OUTW]),
                        op=OP.mult)
                    s1 = wpool.tile([P, F2 // 2], F32, tag="s1")
                    nc.vector.tensor_add(s1[:], w_[:, 0:F2 // 2],
                                         w_[:, F2 // 2:F2])
                    s2 = wpool.tile([P, F2 // 4], F32, tag="s2")
                    nc.vector.tensor_add(s2[:], s1[:, 0:F2 // 4],
                                         s1[:, F2 // 4:F2 // 2])
                    ob = wpool.tile([P, OUTW], F32, tag="ob")
                    nc.vector.tensor_add(ob[:], s2[:, 0:OUTW],
                                         s2[:, OUTW:2 * OUTW])
                    of = wpool.tile([P, OUTW], F32, tag="of")
                    nc.vector.tensor_add(of[:], ob[:], b2mean[:])
                    lo = b * P
                    hi = min(cfg.Nloc, lo + P)
                    if hi > lo:
                        nc.sync.dma_start(d_out[lo:hi, :], of[0:hi - lo, :])

            if dbg:
                for nm, src_t in [("dbg_tab1", d_tab1), ("dbg_er1", d_er1),
                                  ("dbg_hT", d_hT), ("dbg_tab2", d_tab2),
                                  ("dbg_er2", d_er2)] + (
                                      [("dbg_hTg", d_hTg)] if C > 1 else []):
                    dd = nc.dram_tensor(nm, list(src_t.shape), BF16,
                                        kind="ExternalOutput")
                    sl = tuple(slice(None) for _ in src_t.shape)
                    nc.sync.dma_start(dd[sl], src_t[sl])

    nc.compile()
    return nc


# ----------------------------------------------------------------------------
_CACHE = {}


def get_built(src, dst, C=8, cfg=None):
    key = (hash(src.tobytes()), hash(dst.tobytes()), C)
    if key not in _CACHE:
        if cfg is None:
            cfg = GATCfg(C=C)
        idx = prep_indices(src, dst, cfg)
        nc = build_module(cfg)
        _CACHE[key] = (cfg, idx, nc)
    return _CACHE[key]


_EXECC = {}


def _get_exec(key, nc, n_cores):
    """Persistent jit(shard_map(bass_exec)) so repeated kernel() calls skip
    retracing/recompiling (run_bass_kernel_spmd rebuilds its jit per call)."""
    if key in _EXECC:
        return _EXECC[key]
    import jax
    from jax.experimental.shard_map import shard_map
    from jax.sharding import Mesh, NamedSharding, PartitionSpec
    from concourse import bass2jax
    bass2jax.install_neuronx_cc_hook()
    partition_name = (nc.partition_id_tensor.name
                      if nc.partition_id_tensor else None)
    in_names, out_names, out_avals, zero_shapes = [], [], [], []
    for alloc in nc.m.functions[0].allocations:
        if not isinstance(alloc, mybir.MemoryLocationSet):
            continue
        name = alloc.memorylocations[0].name
        if alloc.kind == "ExternalInput":
            if name != partition_name:
                in_names.append(name)
        elif alloc.kind == "ExternalOutput":
            out_names.append(name)
            shape = tuple(alloc.tensor_shape)
            dtype = mybir.dt.np(alloc.dtype)
            out_avals.append(jax.core.ShapedArray(shape, dtype))
            zero_shapes.append((shape, dtype))
    n_params = len(in_names)
    in_names_all = list(in_names) + out_names + (
        [partition_name] if partition_name else [])

    def _body(*args):
        ops = list(args)
        if partition_name:
            ops.append(bass2jax.partition_id_tensor())
        outs = bass2jax._bass_exec_p.bind(
            *ops, out_avals=tuple(out_avals), in_names=tuple(in_names_all),
            out_names=tuple(out_names), lowering_input_output_aliases=(),
            sim_require_finite=True, sim_require_nnan=True, nc=nc)
        return tuple(outs)

    devices = jax.devices()[:n_cores]
    mesh = Mesh(np.asarray(devices), ("core",))
    nout = len(out_names)
    f = jax.jit(shard_map(
        _body, mesh=mesh,
        in_specs=(PartitionSpec("core"),) * (n_params + nout),
        out_specs=(PartitionSpec("core"),) * nout, check_rep=False),
        keep_unused=True)
    sh = NamedSharding(mesh, PartitionSpec("core"))
    ent = dict(f=f, in_names=in_names, out_names=out_names,
               zero_shapes=zero_shapes, sh=sh, argcache=None)
    _EXECC[key] = ent
    return ent


def kernel(**inputs) -> np.ndarray:
    import jax
    src = np.asarray(inputs["src"], np.int32)
    dst = np.asarray(inputs["dst"], np.int32)
    x = np.asarray(inputs["x"])
    base = GATCfg(N=int(x.shape[0]), C=8, IN=int(x.shape[1]))
    cfg, idx, nc = get_built(src, dst, C=8, cfg=base)
    in_maps = host_inputs(inputs, cfg, idx)
    key = (hash(src.tobytes()), hash(dst.tobytes()), cfg.C)
    ent = _get_exec(key, nc, cfg.C)
    C = cfg.C
    concat_in = [np.ascontiguousarray(
        np.concatenate([in_maps[c][nm] for c in range(C)], axis=0))
        for nm in ent["in_names"]]
    hashes = tuple(hash(a.tobytes()) for a in concat_in)
    if ent["argcache"] is None or ent["argcache"][0] != hashes:
        zeros = [np.zeros((C * sh0[0], *sh0[1:]), dt)
                 for sh0, dt in ent["zero_shapes"]]
        args = [jax.device_put(a, ent["sh"]) for a in concat_in + zeros]
        ent["argcache"] = (hashes, args)
    args = ent["argcache"][1]
    outs = ent["f"](*args)
    jax.block_until_ready(outs)
    oi = ent["out_names"].index("out")
    out = np.asarray(outs[oi]).reshape(C, cfg.Nloc, cfg.OUT)
    return out.reshape(-1, cfg.OUT)[:cfg.N].astype(np.float32)


# revision 23
# speedup vs baseline: 1.4115x; 1.4115x over previous
"""2-layer GAT (graph attention) Bass/Tile kernel for Trainium2, 8-core SPMD.

Sharding: nodes partitioned contiguously across cores; edges assigned to the
core owning their dst, sorted by dst, grouped into 128-dst blocks and 128-edge
chunks (uniformly padded so all cores share one SPMD module).

Per core: build the full node feature table (replicated matmul from a
host-transposed xT), rows [feat bf16 | el bf16 | pad] in HBM (256B-multiple
rows for dma_gather).  Edge phase per block: dma_gather of src rows (split
into lo/hi index halves since gather indices are int16), a dense per-block er
load broadcast to edges via the PE-transposed one-hot, then per chunk one PE
matmul accumulating the weighted message sum and the exp-sum into PSUM
(softmax max-subtraction is skipped; |e| is O(1)).  Per-dst normalization
happens after the reduction.  Hidden states are transposed and AllGather'ed
between layers.
"""

import math
import os

import numpy as np

import concourse.bacc as bacc
import concourse.bass as bass
import concourse.mybir as mybir
import concourse.tile as tile
from concourse.masks import make_identity

F32 = mybir.dt.float32
BF16 = mybir.dt.bfloat16
I32 = mybir.dt.int32
I16 = mybir.dt.int16
AF = mybir.ActivationFunctionType
OP = mybir.AluOpType

P = 128
HALF = 32768  # int16 gather index limit


class GATCfg:
    def __init__(self, N=50000, C=8, IN=128, HID=32, HEADS=8, OUT=16, NEG=0.2):
        self.N, self.C, self.IN = N, C, IN
        self.HID, self.HEADS, self.OUT, self.NEG = HID, HEADS, OUT, NEG
        self.HALF = HALF   # int16 gather index split (overridable in tests)
        self.SLO = 0   # lo-half slots per block (set by prep)
        self.SHI = 0   # hi-half slots per block
        self.F1 = HEADS * HID
        self.F2 = HEADS * OUT
        # table rows padded to a 256B multiple (bf16 elements)
        self.ROW1 = 384 if self.F1 + 8 > 256 else 256      # 768B
        self.ROW2 = 256 if self.F2 + 8 > 128 else 128      # 512B
        self.Nloc = (N + C - 1) // C
        self.NB = (self.Nloc + P - 1) // P
        self.Nlp = self.NB * P
        self.NP1 = ((N + 511) // 512) * 512
        self.NP2 = C * self.Nlp

    @property
    def NCt(self):
        return (self.SLO + self.SHI) // P


def _wrap16(vals_slots, NB, S):
    """[NB, S] slot-ordered ints -> [128, NB*S//16] 16-wrapped, replicated 8x."""
    a = vals_slots.reshape(NB, S // 16, 16)
    out = np.zeros((128, NB * (S // 16)), np.int16)
    for b in range(NB):
        blkcols = a[b].T.astype(np.int16)          # [16, S//16]
        for r in range(8):
            out[r * 16:(r + 1) * 16,
                b * (S // 16):(b + 1) * (S // 16)] = blkcols
    return out


def prep_indices(src, dst, cfg):
    """Host index-only preprocessing."""
    C, Nloc, NB = cfg.C, cfg.Nloc, cfg.NB
    src = np.asarray(src).astype(np.int64)
    dst = np.asarray(dst).astype(np.int64)
    core = dst // Nloc
    dloc = dst - core * Nloc
    blk = dloc // P
    dblk = dloc - blk * P
    key = core * NB + blk
    order = np.argsort(key, kind="stable")
    counts = np.bincount(key, minlength=C * NB)
    starts = np.zeros(C * NB + 1, np.int64)
    np.cumsum(counts, out=starts[1:])

    r_of = src // Nloc
    srcp = r_of * cfg.Nlp + (src - r_of * Nloc)   # layer-2 rank-major id

    # per-layer lo/hi counts to size SLO/SHI uniformly
    HALFc = cfg.HALF

    def lohi_max(ids):
        lo_max = hi_max = 0
        for k in range(C * NB):
            e = order[starts[k]:starts[k + 1]]
            n_lo = int((ids[e] < HALFc).sum())
            lo_max = max(lo_max, n_lo)
            hi_max = max(hi_max, e.size - n_lo)
        return lo_max, hi_max

    lo1, hi1 = lohi_max(src)
    lo2, hi2 = lohi_max(srcp)
    SLO = ((max(lo1, lo2, 1) + P - 1) // P) * P
    SHI = ((max(hi1, hi2) + P - 1) // P) * P
    cfg.SLO, cfg.SHI = SLO, SHI
    S = SLO + SHI
    NCt = S // P

    out = {}
    for layer, ids in ((1, src), (2, srcp)):
        idxlo = np.zeros((C, NB, SLO), np.int64)
        idxhi = np.zeros((C, NB, max(SHI, 16)), np.int64)
        dstb = np.full((C, P, NB * NCt), -1.0, np.float32)
        dbf = np.full((C, NB * NCt * P), -1, np.int8)
        for c in range(C):
            for b in range(NB):
                k = c * NB + b
                e = order[starts[k]:starts[k + 1]]
                v = ids[e]
                m = v < HALFc
                elo, ehi = e[m], e[~m]
                idxlo[c, b, :elo.size] = v[m]
                if ehi.size:
                    idxhi[c, b, :ehi.size] = v[~m] - HALFc
                # dst-in-block values at slots [0,nlo) and [SLO, SLO+nhi)
                sl = np.concatenate([np.arange(elo.size),
                                     SLO + np.arange(ehi.size)])
                ee = np.concatenate([elo, ehi])
                dstb[c, sl % P, b * NCt + sl // P] = dblk[ee]
                dbf[c, b * NCt * P + sl] = dblk[ee]
        wlo = np.stack([_wrap16(idxlo[c], NB, SLO) for c in range(C)])
        whi = np.stack([_wrap16(idxhi[c], NB, max(SHI, 16)) for c in range(C)])
        out[f"idxlo{layer}"] = wlo
        out[f"idxhi{layer}"] = whi
        out[f"dstb{layer}"] = dstb
        out[f"dbb{layer}"] = dbf
    return out


def _alr_mat(al, ar, F, H, D):
    m = np.zeros((F, 16), np.float32)
    for h in range(H):
        m[h * D:(h + 1) * D, h] = al[h]
        m[h * D:(h + 1) * D, 8 + h] = ar[h]
    return m


def host_inputs(inputs, cfg, idx):
    x = np.asarray(inputs["x"], np.float32)
    xT = np.zeros((cfg.IN, cfg.NP1), np.float32)
    xT[:, :cfg.N] = np.ascontiguousarray(x.T)
    W1 = np.asarray(inputs["W1"], np.float32)
    W2 = np.asarray(inputs["W2"], np.float32)
    alr1 = _alr_mat(np.asarray(inputs["al1"], np.float32),
                    np.asarray(inputs["ar1"], np.float32),
                    cfg.F1, cfg.HEADS, cfg.HID)
    alr2 = _alr_mat(np.asarray(inputs["al2"], np.float32),
                    np.asarray(inputs["ar2"], np.float32),
                    cfg.F2, cfg.HEADS, cfg.OUT)
    b1 = np.asarray(inputs["b1"], np.float32).reshape(1, cfg.F1)
    b2 = np.asarray(inputs["b2"], np.float32).reshape(1, cfg.F2)

    in_maps = []
    for c in range(cfg.C):
        xTl = np.zeros((cfg.IN, cfg.Nlp), np.float32)
        lo = c * cfg.Nloc
        hi = min(cfg.N, lo + cfg.Nloc)
        xTl[:, :hi - lo] = xT[:, lo:hi]
        in_maps.append({
            "xT": xT, "xTl": xTl,
            "W1": W1, "W1T": np.ascontiguousarray(W1.T),
            "W2": W2, "W2T": np.ascontiguousarray(W2.T),
            "ALR1": alr1, "ALR2": alr2,
            "B1": b1, "B2": b2,
            "idxlo1": idx["idxlo1"][c], "idxhi1": idx["idxhi1"][c],
            "idxlo2": idx["idxlo2"][c], "idxhi2": idx["idxhi2"][c],
            "dstb1": idx["dstb1"][c], "dstb2": idx["dstb2"][c],
            "dbb1": np.ascontiguousarray(
                np.broadcast_to(idx["dbb1"][c][None, :],
                                (128, idx["dbb1"].shape[1]))),
            "dbb2": np.ascontiguousarray(
                np.broadcast_to(idx["dbb2"][c][None, :],
                                (128, idx["dbb2"].shape[1]))),
        })
    return in_maps


def build_module(cfg, dbg=False, skip_cc=False):
    nc = bacc.Bacc("TRN2", target_bir_lowering=False, debug=False,
                   num_devices=cfg.C)
    C, NB, Nlp = cfg.C, cfg.NB, cfg.Nlp
    F1, F2, ROW1, ROW2 = cfg.F1, cfg.F2, cfg.ROW1, cfg.ROW2
    SLO, SHI, NCt = cfg.SLO, cfg.SHI, cfg.NCt
    NLOC, NHIC = SLO // P, SHI // P

    d_xT = nc.dram_tensor("xT", [cfg.IN, cfg.NP1], F32, kind="ExternalInput")
    d_xTl = nc.dram_tensor("xTl", [cfg.IN, Nlp], F32, kind="ExternalInput")
    d_W1 = nc.dram_tensor("W1", [cfg.IN, F1], F32, kind="ExternalInput")
    d_W1T = nc.dram_tensor("W1T", [F1, cfg.IN], F32, kind="ExternalInput")
    d_W2 = nc.dram_tensor("W2", [F1, F2], F32, kind="ExternalInput")
    d_W2T = nc.dram_tensor("W2T", [F2, F1], F32, kind="ExternalInput")
    d_ALR1 = nc.dram_tensor("ALR1", [F1, 16], F32, kind="ExternalInput")
    d_ALR2 = nc.dram_tensor("ALR2", [F2, 16], F32, kind="ExternalInput")
    d_B1 = nc.dram_tensor("B1", [1, F1], F32, kind="ExternalInput")
    d_B2 = nc.dram_tensor("B2", [1, F2], F32, kind="ExternalInput")
    d_il1 = nc.dram_tensor("idxlo1", [P, NB * (SLO // 16)], I16,
                           kind="ExternalInput")
    d_ih1 = nc.dram_tensor("idxhi1", [P, NB * (max(SHI, 16) // 16)], I16,
                           kind="ExternalInput")
    d_il2 = nc.dram_tensor("idxlo2", [P, NB * (SLO // 16)], I16,
                           kind="ExternalInput")
    d_ih2 = nc.dram_tensor("idxhi2", [P, NB * (max(SHI, 16) // 16)], I16,
                           kind="ExternalInput")
    d_db1 = nc.dram_tensor("dstb1", [P, NB * NCt], F32, kind="ExternalInput")
    d_db2 = nc.dram_tensor("dstb2", [P, NB * NCt], F32, kind="ExternalInput")
    I8 = mybir.dt.int8
    d_dbb1 = nc.dram_tensor("dbb1", [P, NB * NCt * P], I8,
                            kind="ExternalInput")
    d_dbb2 = nc.dram_tensor("dbb2", [P, NB * NCt * P], I8,
                            kind="ExternalInput")
    d_out = nc.dram_tensor("out", [cfg.Nloc, cfg.OUT], F32,
                           kind="ExternalOutput")

    d_tab1 = nc.dram_tensor("tab1", [cfg.NP1, ROW1], BF16, kind="Internal")
    d_tab2 = nc.dram_tensor("tab2", [cfg.NP2, ROW2], BF16, kind="Internal")
    d_er1 = nc.dram_tensor("er1", [Nlp, 8], BF16, kind="Internal")
    d_er2 = nc.dram_tensor("er2", [Nlp, 8], BF16, kind="Internal")
    d_hT = nc.dram_tensor("hT", [2, P, Nlp], BF16, kind="Internal")
    d_hTg = None
    if C > 1:
        d_hTg = nc.dram_tensor("hTg", [C, 2, P, Nlp], BF16, kind="Internal",
                               addr_space="Shared" if C > 4 else "Local")

    with tile.TileContext(nc) as tc:
        with (
            tc.tile_pool(name="const", bufs=1) as cpool,
            tc.tile_pool(name="work", bufs=3) as wpool,
            tc.tile_pool(name="gath", bufs=2) as gpool,
        ):
            # ---------------- constants ----------------
            iota_i = cpool.tile([P, P], I32)
            nc.gpsimd.iota(iota_i[:], pattern=[[1, P]], base=0,
                           channel_multiplier=0)
            iota_bf = cpool.tile([P, P], BF16)
            nc.vector.tensor_copy(iota_bf[:], iota_i[:])
            ident = cpool.tile([P, P], BF16)
            make_identity(nc, ident[:])
            iota_ci = cpool.tile([P, 1], I32)
            nc.gpsimd.iota(iota_ci[:], pattern=[[1, 1]], base=0,
                           channel_multiplier=1)
            iota_cf = cpool.tile([P, 1], F32)
            nc.vector.tensor_copy(iota_cf[:], iota_ci[:])
            ones1 = cpool.tile([1, P], BF16)
            nc.vector.memset(ones1[:], 1.0)

            rhs1 = cpool.tile([P, F1 + 16], BF16)
            rhs2 = cpool.tile([P, 2, F2 + 16], BF16)
            b1row = cpool.tile([P, F1], F32)
            b2mean = cpool.tile([P, cfg.OUT], F32)

            with tc.tile_pool(name="cps", bufs=2, space="PSUM") as cps:
                w1t_a = cpool.tile([P, cfg.IN], F32, name="w1t_a")
                w1t_b = cpool.tile([P, cfg.IN], F32, name="w1t_b")
                nc.sync.dma_start(w1t_a[:], d_W1T[0:P, :])
                nc.sync.dma_start(w1t_b[:], d_W1T[P:2 * P, :])
                alr1_s = cpool.tile([P, 2, 16], F32)
                nc.sync.dma_start(alr1_s[:, 0, :], d_ALR1[0:P, :])
                nc.sync.dma_start(alr1_s[:, 1, :], d_ALR1[P:F1, :])
                ps_wel = cps.tile([P, 16], F32, tag="cwel")
                nc.tensor.matmul(ps_wel[:], lhsT=w1t_a[:], rhs=alr1_s[:, 0, :],
                                 start=True, stop=False)
                nc.tensor.matmul(ps_wel[:], lhsT=w1t_b[:], rhs=alr1_s[:, 1, :],
                                 start=False, stop=True)
                w1_f = cpool.tile([P, F1], F32, name="w1_f")
                nc.sync.dma_start(w1_f[:], d_W1[:, :])
                nc.vector.tensor_copy(rhs1[:, 0:F1], w1_f[:])
                nc.vector.tensor_copy(rhs1[:, F1:F1 + 16], ps_wel[:])

                w2t_s = cpool.tile([F2, F1], F32)
                nc.sync.dma_start(w2t_s[:], d_W2T[:, :])
                alr2_s = cpool.tile([F2, 16], F32)
                nc.sync.dma_start(alr2_s[:], d_ALR2[:, :])
                ps_wel2 = cps.tile([P, 2, 16], F32, tag="cwel2")
                for q in range(2):
                    nc.tensor.matmul(ps_wel2[:, q, :],
                                     lhsT=w2t_s[:, q * P:(q + 1) * P],
                                     rhs=alr2_s[:], start=True, stop=True)
                w2_f = cpool.tile([P, 2, F2], F32)
                nc.sync.dma_start(w2_f[:, 0, :], d_W2[0:P, :])
                nc.sync.dma_start(w2_f[:, 1, :], d_W2[P:F1, :])
                for q in range(2):
                    nc.vector.tensor_copy(rhs2[:, q, 0:F2], w2_f[:, q, :])
                    nc.vector.tensor_copy(rhs2[:, q, F2:F2 + 16],
                                          ps_wel2[:, q, :])

                b1_r = cpool.tile([1, F1], BF16, name="b1_r")
                b1_f = cpool.tile([1, F1], F32, name="b1_f")
                nc.sync.dma_start(b1_f[:], d_B1[:, :])
                nc.vector.tensor_copy(b1_r[:], b1_f[:])
                ps_b1 = cps.tile([P, F1], F32, tag="cb1")
                nc.tensor.matmul(ps_b1[:], lhsT=ones1[:], rhs=b1_r[:],
                                 start=True, stop=True)
                nc.vector.tensor_copy(b1row[:], ps_b1[:])

                b2_r = cpool.tile([1, F2], BF16, name="b2_r")
                b2_f = cpool.tile([1, F2], F32, name="b2_f")
                nc.sync.dma_start(b2_f[:], d_B2[:, :])
                nc.vector.tensor_copy(b2_r[:], b2_f[:])
                ps_b2 = cps.tile([P, F2], F32, tag="cb2")
                nc.tensor.matmul(ps_b2[:], lhsT=ones1[:], rhs=b2_r[:],
                                 start=True, stop=True)
                b2full = cpool.tile([P, F2], F32)
                nc.vector.tensor_copy(b2full[:], ps_b2[:])
                b2h = cpool.tile([P, F2 // 2], F32)
                nc.vector.tensor_add(b2h[:], b2full[:, 0:F2 // 2],
                                     b2full[:, F2 // 2:F2])
                b2q = cpool.tile([P, F2 // 4], F32)
                nc.vector.tensor_add(b2q[:], b2h[:, 0:F2 // 4],
                                     b2h[:, F2 // 4:F2 // 2])
                b2s = cpool.tile([P, cfg.OUT], F32)
                nc.vector.tensor_add(b2s[:], b2q[:, 0:cfg.OUT],
                                     b2q[:, cfg.OUT:2 * cfg.OUT])
                nc.scalar.mul(b2mean[:], b2s[:], 0.125)

            # ---------------- layer-1 table ----------------
            with tc.tile_pool(name="t1ps", bufs=3, space="PSUM") as t1ps:
                nt1 = cfg.NP1 // P
                for t in range(nt1):
                    xt = wpool.tile([P, P], F32, tag="xt")
                    nc.sync.dma_start(xt[:], d_xT[:, t * P:(t + 1) * P])
                    xb = wpool.tile([P, P], BF16, tag="xb")
                    (nc.vector.tensor_copy if t % 2 == 0
                     else nc.scalar.copy)(xb[:], xt[:])
                    ps = t1ps.tile([P, F1 + 16], F32, tag="tbl")
                    nc.tensor.matmul(ps[:], lhsT=xb[:], rhs=rhs1[:],
                                     start=True, stop=True)
                    fe = wpool.tile([P, F1 + 8], BF16, tag="fe")
                    (nc.scalar.copy if t % 2 == 0
                     else nc.vector.tensor_copy)(fe[:], ps[:, 0:F1 + 8])
                    nc.sync.dma_start(d_tab1[t * P:(t + 1) * P, 0:F1 + 8],
                                      fe[:])

                for t in range(NB):
                    xt = wpool.tile([P, P], F32, tag="xt")
                    nc.sync.dma_start(xt[:], d_xTl[:, t * P:(t + 1) * P])
                    xb = wpool.tile([P, P], BF16, tag="xb")
                    nc.vector.tensor_copy(xb[:], xt[:])
                    ps = t1ps.tile([P, 16], F32, tag="er")
                    nc.tensor.matmul(ps[:], lhsT=xb[:],
                                     rhs=rhs1[:, F1:F1 + 16],
                                     start=True, stop=True)
                    erb = wpool.tile([P, 8], BF16, tag="erb")
                    nc.vector.tensor_copy(erb[:], ps[:, 8:16])
                    nc.sync.dma_start(d_er1[t * P:(t + 1) * P, :], erb[:])

            # ---------------- edge phase (shared) ----------------
            def edge_phase(layer, pspool, tps):
                F = F1 if layer == 1 else F2
                ROW = ROW1 if layer == 1 else ROW2
                tab = d_tab1 if layer == 1 else d_tab2
                ertab = d_er1 if layer == 1 else d_er2
                d_il = d_il1 if layer == 1 else d_il2
                d_ih = d_ih1 if layer == 1 else d_ih2
                d_db = d_db1 if layer == 1 else d_db2
                d_dbb = d_dbb1 if layer == 1 else d_dbb2
                CL = SLO // 16
                CH = max(SHI, 16) // 16
                PIECE = 768  # max slots per dma_gather (64-desc/lane packet cap)
                for b in range(NB):
                    il = wpool.tile([P, CL], I16, tag="il")
                    nc.sync.dma_start(il[:], d_il[:, b * CL:(b + 1) * CL])
                    Glo = gpool.tile([P, NLOC, ROW], BF16, tag=f"Glo{layer}")
                    for s0 in ([] if "nogather" in os.environ.get(
                            "GAT_ABLATE", "") else range(0, SLO, PIECE)):
                        n = min(PIECE, SLO - s0)
                        nc.gpsimd.dma_gather(
                            out_ap=Glo[:, s0 // P:(s0 + n) // P, :],
                            in_ap=tab[:, :],
                            idxs_ap=il[:, s0 // 16:(s0 + n) // 16],
                            num_idxs=n, num_idxs_reg=n, elem_size=ROW)
                    if SHI > 0:
                        ih = wpool.tile([P, CH], I16, tag="ih")
                        nc.sync.dma_start(ih[:], d_ih[:, b * CH:(b + 1) * CH])
                        Ghi = gpool.tile([P, NHIC, ROW], BF16,
                                         tag=f"Ghi{layer}")
                        for s0 in ([] if "nogather" in os.environ.get(
                                "GAT_ABLATE", "") else range(0, SHI, PIECE)):
                            n = min(PIECE, SHI - s0)
                            nc.gpsimd.dma_gather(
                                out_ap=Ghi[:, s0 // P:(s0 + n) // P, :],
                                in_ap=tab[cfg.HALF:, :],
                                idxs_ap=ih[:, s0 // 16:(s0 + n) // 16],
                                num_idxs=n, num_idxs_reg=n, elem_size=ROW)
                    db = wpool.tile([P, NCt], F32, tag="db")
                    nc.sync.dma_start(db[:], d_db[:, b * NCt:(b + 1) * NCt])
                    dbb = wpool.tile([P, NCt * P], mybir.dt.int8, tag="dbb")
                    nc.sync.dma_start(
                        dbb[:], d_dbb[:, b * NCt * P:(b + 1) * NCt * P])
                    erblk = wpool.tile([P, 8], BF16, tag="erblk")
                    nc.sync.dma_start(erblk[:], ertab[b * P:(b + 1) * P, :])

                    ps = pspool.tile([P, F + 8], F32, tag="eps")
                    # block-batched attention scalars
                    erps = tps.tile([P, NCt, 8], F32, tag="erps")
                    ohTs = []
                    for c in range(NCt):
                        ohT = wpool.tile([P, P], BF16, tag=f"ohT{c % 2}")
                        nc.vector.tensor_scalar(
                            ohT[:], dbb[:, c * P:(c + 1) * P],
                            iota_cf[:, 0:1], None, op0=OP.is_equal)
                        nc.tensor.matmul(erps[:, c, :], lhsT=ohT[:],
                                         rhs=erblk[:], start=True, stop=True)
                    Gel = (Glo[:, :, F:F + 8] if SHI == 0 else None)
                    e_all = wpool.tile([P, NCt, 8], F32, tag="e_all")
                    if SHI == 0:
                        nc.vector.tensor_tensor(out=e_all[:], in0=Gel,
                                                in1=erps[:], op=OP.add)
                    else:
                        nc.vector.tensor_tensor(
                            out=e_all[:, 0:NLOC, :],
                            in0=Glo[:, :, F:F + 8],
                            in1=erps[:, 0:NLOC, :], op=OP.add)
                        nc.vector.tensor_tensor(
                            out=e_all[:, NLOC:NCt, :],
                            in0=Ghi[:, :, F:F + 8],
                            in1=erps[:, NLOC:NCt, :], op=OP.add)
                    es_a = wpool.tile([P, NCt, 8], F32, tag="es_a")
                    nc.vector.tensor_scalar(es_a[:], e_all[:], cfg.NEG, None,
                                            op0=OP.mult)
                    nc.vector.tensor_tensor(out=es_a[:], in0=e_all[:],
                                            in1=es_a[:], op=OP.max)
                    MSG = gpool.tile([P, NCt, F + 8], BF16,
                                     tag=f"MSG{layer}")
                    nc.scalar.activation(MSG[:, :, F:F + 8], es_a[:], AF.Exp)

                    def _mult(mout, gin, exin):
                        nc.vector.tensor_tensor(
                            out=mout.rearrange("p c (h d) -> p c h d", h=8),
                            in0=gin.rearrange("p c (h d) -> p c h d", h=8),
                            in1=exin.rearrange("p c (h one) -> p c h one",
                                               one=1)
                                .to_broadcast([P, exin.shape[1], 8, F // 8]),
                            op=OP.mult)

                    if SHI == 0:
                        _mult(MSG[:, :, 0:F], Glo[:, :, 0:F],
                              MSG[:, :, F:F + 8])
                    else:
                        _mult(MSG[:, 0:NLOC, 0:F], Glo[:, :, 0:F],
                              MSG[:, 0:NLOC, F:F + 8])
                        _mult(MSG[:, NLOC:NCt, 0:F], Ghi[:, :, 0:F],
                              MSG[:, NLOC:NCt, F:F + 8])
                    for c in range(NCt):
                        oh = wpool.tile([P, P], BF16, tag="oh")
                        nc.vector.tensor_scalar(oh[:], iota_bf[:],
                                                db[:, c:c + 1], None,
                                                op0=OP.is_equal)
                        nc.tensor.matmul(ps[:], lhsT=oh[:], rhs=MSG[:, c, :],
                                         start=(c == 0), stop=(c == NCt - 1))

                    esum = wpool.tile([P, 8], F32, tag="esum")
                    nc.vector.tensor_scalar(esum[:], ps[:, F:F + 8], 1e-30,
                                            None, op0=OP.max)
                    inv = wpool.tile([P, 8], F32, tag="inv")
                    nc.vector.reciprocal(inv[:], esum[:])
                    yield b, ps, inv

            # ---------------- layer-1 edges + hT ----------------
            with tc.tile_pool(name="e1ps", bufs=2, space="PSUM") as e1ps, \
                 tc.tile_pool(name="tps", bufs=2, space="PSUM") as tps:
                for b, ps, inv in edge_phase(1, e1ps, tps):
                    z = wpool.tile([P, F1], F32, tag="z")
                    nc.vector.tensor_tensor(
                        out=z[:].rearrange("p (h d) -> p h d", h=8),
                        in0=ps[:, 0:F1].rearrange("p (h d) -> p h d", h=8),
                        in1=inv[:].rearrange("p (h one) -> p h one", one=1)
                            .to_broadcast([P, 8, cfg.HID]),
                        op=OP.mult)
                    nc.vector.tensor_add(z[:], z[:], b1row[:])
                    zm = wpool.tile([P, F1], F32, tag="zm")
                    nc.vector.tensor_scalar(zm[:], z[:], 0.0, None, op0=OP.min)
                    zp = wpool.tile([P, F1], F32, tag="zp")
                    nc.vector.tensor_scalar(zp[:], z[:], 0.0, None, op0=OP.max)
                    q_ = wpool.tile([P, F1], F32, tag="q_")
                    nc.scalar.activation(q_[:], zm[:], AF.Exp)
                    s_ = wpool.tile([P, F1], F32, tag="s_")
                    nc.vector.tensor_add(s_[:], zp[:], q_[:])
                    hb = wpool.tile([P, F1], BF16, tag="hb")
                    nc.vector.tensor_scalar(hb[:], s_[:], -1.0, None,
                                            op0=OP.add)
                    for q in range(2):
                        pst = tps.tile([P, P], BF16, tag="pst")
                        nc.tensor.transpose(pst[:], hb[:, q * P:(q + 1) * P],
                                            ident[:])
                        htp = wpool.tile([P, P], BF16, tag="htp")
                        (nc.vector.tensor_copy if q == 0
                         else nc.scalar.copy)(htp[:], pst[:])
                        nc.sync.dma_start(d_hT[q, :, b * P:(b + 1) * P],
                                          htp[:])

            # ---------------- allgather ----------------
            if C > 1 and not skip_cc:
                nc.gpsimd.collective_compute(
                    "AllGather", OP.bypass,
                    replica_groups=[list(range(C))],
                    ins=[d_hT[:, :, :]],
                    outs=[d_hTg[:, :, :, :]],
                )

            # ---------------- layer-2 table ----------------
            with tc.tile_pool(name="t2ps", bufs=3, space="PSUM") as t2ps:
                for r in range(C):
                    for t in range(NB):
                        ht_a = wpool.tile([P, P], BF16, tag="ht_a")
                        ht_b = wpool.tile([P, P], BF16, tag="ht_b")
                        if C > 1:
                            nc.sync.dma_start(
                                ht_a[:], d_hTg[r, 0, :, t * P:(t + 1) * P])
                            nc.sync.dma_start(
                                ht_b[:], d_hTg[r, 1, :, t * P:(t + 1) * P])
                        else:
                            nc.sync.dma_start(ht_a[:],
                                              d_hT[0, :, t * P:(t + 1) * P])
                            nc.sync.dma_start(ht_b[:],
                                              d_hT[1, :, t * P:(t + 1) * P])
                        ps = t2ps.tile([P, F2 + 16], F32, tag="tbl2")
                        nc.tensor.matmul(ps[:], lhsT=ht_a[:], rhs=rhs2[:, 0, :],
                                         start=True, stop=False)
                        nc.tensor.matmul(ps[:], lhsT=ht_b[:], rhs=rhs2[:, 1, :],
                                         start=False, stop=True)
                        fe = wpool.tile([P, F2 + 8], BF16, tag="fe2")
                        (nc.scalar.copy if t % 2 == 0
                         else nc.vector.tensor_copy)(fe[:], ps[:, 0:F2 + 8])
                        row0 = (r * NB + t) * P
                        nc.sync.dma_start(d_tab2[row0:row0 + P, 0:F2 + 8],
                                          fe[:])

                for t in range(NB):
                    ht_a = wpool.tile([P, P], BF16, tag="ht_a")
                    nc.sync.dma_start(ht_a[:], d_hT[0, :, t * P:(t + 1) * P])
                    ht_b = wpool.tile([P, P], BF16, tag="ht_b")
                    nc.sync.dma_start(ht_b[:], d_hT[1, :, t * P:(t + 1) * P])
                    ps = t2ps.tile([P, 16], F32, tag="er")
                    nc.tensor.matmul(ps[:], lhsT=ht_a[:],
                                     rhs=rhs2[:, 0, F2:F2 + 16],
                                     start=True, stop=False)
                    nc.tensor.matmul(ps[:], lhsT=ht_b[:],
                                     rhs=rhs2[:, 1, F2:F2 + 16],
                                     start=False, stop=True)
                    erb = wpool.tile([P, 8], BF16, tag="erb")
                    nc.vector.tensor_copy(erb[:], ps[:, 8:16])
                    nc.sync.dma_start(d_er2[t * P:(t + 1) * P, :], erb[:])

            # ---------------- layer-2 edges + output ----------------
            OUTW = cfg.OUT
            with tc.tile_pool(name="e2ps", bufs=2, space="PSUM") as e2ps, \
                 tc.tile_pool(name="tps2", bufs=2, space="PSUM") as tps2:
                for b, ps, inv in edge_phase(2, e2ps, tps2):
                    inv8 = wpool.tile([P, 8], F32, tag="inv8")
                    nc.scalar.mul(inv8[:], inv[:], 0.125)
                    w_ = wpool.tile([P, F2], F32, tag="w_")
                    nc.vector.tensor_tensor(
                        out=w_[:].rearrange("p (h d) -> p h d", h=8),
                        in0=ps[:, 0:F2].rearrange("p (h d) -> p h d", h=8),
                        in1=inv8[:].rearrange("p (h one) -> p h one", one=1)
                            .to_broadcast([P, 8, OUTW]),
                        op=OP.mult)
                    s1 = wpool.tile([P, F2 // 2], F32, tag="s1")
                    nc.vector.tensor_add(s1[:], w_[:, 0:F2 // 2],
                                         w_[:, F2 // 2:F2])
                    s2 = wpool.tile([P, F2 // 4], F32, tag="s2")
                    nc.vector.tensor_add(s2[:], s1[:, 0:F2 // 4],
                                         s1[:, F2 // 4:F2 // 2])
                    ob = wpool.tile([P, OUTW], F32, tag="ob")
                    nc.vector.tensor_add(ob[:], s2[:, 0:OUTW],
                                         s2[:, OUTW:2 * OUTW])
                    of = wpool.tile([P, OUTW], F32, tag="of")
                    nc.vector.tensor_add(of[:], ob[:], b2mean[:])
                    lo = b * P
                    hi = min(cfg.Nloc, lo + P)
                    if hi > lo:
                        nc.sync.dma_start(d_out[lo:hi, :], of[0:hi - lo, :])

            if dbg:
                for nm, src_t in [("dbg_tab1", d_tab1), ("dbg_er1", d_er1),
                                  ("dbg_hT", d_hT), ("dbg_tab2", d_tab2),
                                  ("dbg_er2", d_er2)] + (
                                      [("dbg_hTg", d_hTg)] if C > 1 else []):
                    dd = nc.dram_tensor(nm, list(src_t.shape), BF16,
                                        kind="ExternalOutput")
                    sl = tuple(slice(None) for _ in src_t.shape)
                    nc.sync.dma_start(dd[sl], src_t[sl])

    nc.compile()
    return nc


# ----------------------------------------------------------------------------
_CACHE = {}


def get_built(src, dst, C=8, cfg=None):
    key = (hash(src.tobytes()), hash(dst.tobytes()), C)
    if key not in _CACHE:
        if cfg is None:
            cfg = GATCfg(C=C)
        idx = prep_indices(src, dst, cfg)
        nc = build_module(cfg)
        _CACHE[key] = (cfg, idx, nc)
    return _CACHE[key]


_EXECC = {}


def _get_exec(key, nc, n_cores):
    """Persistent jit(shard_map(bass_exec)) so repeated kernel() calls skip
    retracing/recompiling (run_bass_kernel_spmd rebuilds its jit per call)."""
    if key in _EXECC:
        return _EXECC[key]
    import jax
    from jax.experimental.shard_map import shard_map
    from jax.sharding import Mesh, NamedSharding, PartitionSpec
    from concourse import bass2jax
    bass2jax.install_neuronx_cc_hook()
    partition_name = (nc.partition_id_tensor.name
                      if nc.partition_id_tensor else None)
    in_names, out_names, out_avals, zero_shapes = [], [], [], []
    for alloc in nc.m.functions[0].allocations:
        if not isinstance(alloc, mybir.MemoryLocationSet):
            continue
        name = alloc.memorylocations[0].name
        if alloc.kind == "ExternalInput":
            if name != partition_name:
                in_names.append(name)
        elif alloc.kind == "ExternalOutput":
            out_names.append(name)
            shape = tuple(alloc.tensor_shape)
            dtype = mybir.dt.np(alloc.dtype)
            out_avals.append(jax.core.ShapedArray(shape, dtype))
            zero_shapes.append((shape, dtype))
    n_params = len(in_names)
    in_names_all = list(in_names) + out_names + (
        [partition_name] if partition_name else [])

    def _body(*args):
        ops = list(args)
        if partition_name:
            ops.append(bass2jax.partition_id_tensor())
        outs = bass2jax._bass_exec_p.bind(
            *ops, out_avals=tuple(out_avals), in_names=tuple(in_names_all),
            out_names=tuple(out_names), lowering_input_output_aliases=(),
            sim_require_finite=True, sim_require_nnan=True, nc=nc)
        return tuple(outs)

    devices = jax.devices()[:n_cores]
    mesh = Mesh(np.asarray(devices), ("core",))
    nout = len(out_names)
    f = jax.jit(shard_map(
        _body, mesh=mesh,
        in_specs=(PartitionSpec("core"),) * (n_params + nout),
        out_specs=(PartitionSpec("core"),) * nout, check_rep=False),
        keep_unused=True)
    sh = NamedSharding(mesh, PartitionSpec("core"))
    ent = dict(f=f, in_names=in_names, out_names=out_names,
               zero_shapes=zero_shapes, sh=sh, argcache=None)
    _EXECC[key] = ent
    return ent


def kernel(**inputs) -> np.ndarray:
    import jax
    src = np.asarray(inputs["src"], np.int32)
    dst = np.asarray(inputs["dst"], np.int32)
    x = np.asarray(inputs["x"])
    base = GATCfg(N=int(x.shape[0]), C=8, IN=int(x.shape[1]))
    cfg, idx, nc = get_built(src, dst, C=8, cfg=base)
    in_maps = host_inputs(inputs, cfg, idx)
    key = (hash(src.tobytes()), hash(dst.tobytes()), cfg.C)
    ent = _get_exec(key, nc, cfg.C)
    C = cfg.C
    concat_in = [np.ascontiguousarray(
        np.concatenate([in_maps[c][nm] for c in range(C)], axis=0))
        for nm in ent["in_names"]]
    hashes = tuple(hash(a.tobytes()) for a in concat_in)
    if ent["argcache"] is None or ent["argcache"][0] != hashes:
        zeros = [np.zeros((C * sh0[0], *sh0[1:]), dt)
                 for sh0, dt in ent["zero_shapes"]]
        args = [jax.device_put(a, ent["sh"]) for a in concat_in + zeros]
        ent["argcache"] = (hashes, args)
    args = ent["argcache"][1]
    outs = ent["f"](*args)
    jax.block_until_ready(outs)
    oi = ent["out_names"].index("out")
    out = np.asarray(outs[oi]).reshape(C, cfg.Nloc, cfg.OUT)
    return out.reshape(-1, cfg.OUT)[:cfg.N].astype(np.float32)
